# revision 1
# baseline (speedup 1.0000x reference)
"""APPNP GNN kernel for 8 Trainium2 NeuronCores.

h0 = MLP(x); h <- 0.9 * D^-1/2 (A+I) D^-1/2 h + 0.1*h0 (10 steps); log_softmax.

Distribution: nodes permuted + bin-packed into degree-balanced groups of <=64
destinations (<=2048 in-edges), 200 groups/core (12800 slots/core). All cores
run one SPMD program; per-core structure lives in data tables.

Per iteration each core:
  - gathers bf16 source rows g[src] for its edges with `dma_gather`
    (int16 indices => the 102400-row table is processed as 4 chunks of 25600;
    each group's slots are laid out as 4 chunk-blocks of 5x128 slots),
  - segment-sums the messages into PSUM with one [128x64] one-hot matmul per
    128-edge tile (one-hot built on DVE: iota ramp vs per-slot dst index),
  - blends with the self-loop term and alpha*h0,
  - AllGathers the rescaled g = dinv*h (rows padded to 256B for the gather
    stride) for the next round.
MLP on pre-transposed bf16 x^T; log_softmax on device. Host preprocessing is
numpy-only.
"""

import numpy as np
import ml_dtypes

import concourse.bass as bass
import concourse.bacc as bacc
import concourse.tile as tile
import concourse.mybir as mybir
from concourse import ap_utils
from concourse.bass import MemorySpace
from concourse.bass_utils import run_bass_kernel_spmd

# ---------------------------------------------------------------- config ----
NC = 8          # cores
N = 100000      # nodes
F = 500         # input features
H = 64          # hidden
C = 40          # classes
E = 3200000     # edges
KSTEPS = 10
ALPHA = 0.1

W = 64          # dst nodes per group
GPC = 200       # groups per core
QT = 5          # 128-slot tiles per (group, chunk) block
NCHUNK = 4      # int16 src chunks
TPG = NCHUNK * QT            # tiles per group (20)
SPG = TPG * 128              # slots per group (2560)
NLOC = GPC * W               # node slots per core (12800)
SB = NLOC // 128             # super-blocks per core (100)
NG = NC * NLOC               # g_full rows (102400)
CHUNK = NG // NCHUNK         # rows per chunk (25600)
FPAD = 512
KT = FPAD // 128
GROW = 128                   # padded g row (elements, bf16) => 256B stride
TT_B = 16                    # tiles per one-hot-build DVE instruction

BF16 = mybir.dt.bfloat16
F32 = mybir.dt.float32
I16 = mybir.dt.int16


def exact_div(a, b):
    assert a % b == 0
    return a // b


def dma_gather_raw(gp, out_ap, in_ap, idxs_ap, num_idxs, elem_size, elem_step,
                   num_idxs_reg=None):
    """dma_gather without the elem_size%256 restriction (stride still 256B).

    num_idxs_reg is the live descriptor count on the Q7 (min-capped by the
    static num_idxs); it must equal the number of valid leading indices."""
    if num_idxs_reg is None:
        num_idxs_reg = num_idxs
    assert idxs_ap.dtype == mybir.dt.int16
    assert in_ap.dtype == out_ap.dtype
    assert in_ap.space == MemorySpace.DRAM
    assert idxs_ap.space == MemorySpace.SBUF
    assert out_ap.space == MemorySpace.SBUF
    assert ap_utils.ap_is_contiguous(out_ap.ap[1:])
    assert ap_utils.ap_is_contiguous(idxs_ap.ap[1:])
    assert in_ap.ap[-1][1] == out_ap.ap[-1][1] == elem_size
    assert out_ap.ap[0][1] * out_ap.ap[1][1] == ((num_idxs + 127) // 128) * 128
    assert in_ap.ap[0][0] == elem_step
    stride_bytes = elem_step * mybir.dt.size(in_ap.dtype)
    stride_bytes_256 = exact_div(stride_bytes, 256)
    _in_ap = gp.lower_ap_dma(in_ap, for_custom_bir_dma=True)
    _idxs_ap = gp.lower_ap(idxs_ap)
    _out_ap = gp.lower_ap(out_ap)
    return gp.add_instruction(
        mybir.InstDMAGatherAnt(
            name=gp.bass.get_next_instruction_name(),
            ins=[*_in_ap, _idxs_ap, gp.lower_val_access(gp.to_reg(num_idxs_reg))],
            outs=[_out_ap],
            transpose=False,
            num_idxs=num_idxs,
            elem_size=elem_size,
            stride_bytes_256=stride_bytes_256,
            gen_mode=0,
            single_packet=True,
            queue_num=0,
            sbuf_tokens_per_rank=0,
            sbuf_free_dim_per_rank=0,
            sbuf_free_dim_pad_per_rank=0,
            sbuf_byte_offset=0,
        )
    )


# ---------------------------------------------------------- preprocessing ----
def _pack_groups(deg, n_groups, cap_nodes, cap_edges):
    import heapq

    n = deg.shape[0]
    order = np.argsort(-deg, kind="stable")
    heap = [(0, g) for g in range(n_groups)]
    heapq.heapify(heap)
    nodes_in = np.zeros(n_groups, np.int64)
    group_of = np.empty(n, np.int64)
    pos_of = np.empty(n, np.int64)
    for node in order:
        d = int(deg[node])
        while True:
            if not heap:
                raise RuntimeError("group packing failed")
            esum, g = heapq.heappop(heap)
            if nodes_in[g] >= cap_nodes:
                continue
            if esum + d > cap_edges:
                raise RuntimeError(f"packing: min sum {esum} + {d} > {cap_edges}")
            group_of[node] = g
            pos_of[node] = nodes_in[g]
            nodes_in[g] += 1
            heapq.heappush(heap, (esum + d, g))
            break
    return group_of, pos_of


def _preprocess(x, W1, b1, W2, b2, edge_index):
    x = np.asarray(x, np.float32)
    W1 = np.asarray(W1, np.float32)
    b1 = np.asarray(b1, np.float32)
    W2 = np.asarray(W2, np.float32)
    b2 = np.asarray(b2, np.float32)
    ei = np.asarray(edge_index)
    src, dst = ei[0].astype(np.int64), ei[1].astype(np.int64)

    deg = np.bincount(dst, minlength=N).astype(np.int64)
    group_of, pos_of = _pack_groups(deg, NC * GPC, W, QT * 128 * NCHUNK)
    new_id = group_of * W + pos_of
    orig_of = np.full(NG, -1, np.int64)
    orig_of[new_id] = np.arange(N)

    # --- edge -> slot tables ---
    src_n = new_id[src]
    dst_n = new_id[dst]
    g_e = dst_n // W                       # global group
    w_e = (dst_n % W).astype(np.float32)   # within-group dst index
    c_e = src_n // CHUNK                   # src chunk
    key = g_e * NCHUNK + c_e
    order = np.argsort(key, kind="stable")
    key_s = key[order]
    src_s = src_n[order]
    w_s = w_e[order]
    starts = np.searchsorted(key_s, np.arange(NC * GPC * NCHUNK))
    pos = np.arange(src_s.shape[0]) - starts[key_s]
    assert pos.max() < QT * 128, f"chunk-block overflow: {pos.max()}"
    # slot position: group g, chunk c, tile t = pos//128, lane = pos%128
    gl = (key_s // NCHUNK) % GPC
    core_e = key_s // (GPC * NCHUNK)
    ch = key_s % NCHUNK
    tile_g = gl * TPG + ch * QT + pos // 128   # tile index within core [0, GPC*TPG)
    lane = pos % 128

    # dstrel table [core][128][GPC*TPG]
    dstrel = np.full((NC, 128, GPC * TPG), -1.0, np.float32)
    dstrel[core_e, lane, tile_g] = w_s

    # idx table: per call (group, chunk) = 640 indices, wrapped int16.
    # call k = gl*NCHUNK + ch; index i = pos; value = src - ch*CHUNK
    # layout [16, calls * (QT*128//16)] -> tiled x8 partitions
    CPC = GPC * NCHUNK                 # calls per core (800)
    COLS = QT * 128 // 16              # idx columns per call (40)
    idxs = np.zeros((NC, 16, CPC * COLS), np.int16)
    call = gl * NCHUNK + ch
    rel = (src_s - ch * CHUNK).astype(np.int16)
    idxs[core_e, pos % 16, call * COLS + pos // 16] = rel
    # static per-call index count: max real count over cores (SPMD => one
    # program), rounded to the 16-index granule the Q7 processes
    gcnt = np.zeros((NC, CPC), np.int64)
    np.add.at(gcnt, (core_e, call), 1)
    call_nidx = np.maximum(16, (gcnt.max(axis=0) + 15) // 16 * 16).astype(np.int64)

    # --- per-node scalars packed [128, 3*SB] ---
    dinv = np.zeros(NG, np.float32)
    real = orig_of >= 0
    dinv[real] = 1.0 / np.sqrt(deg[orig_of[real]] + 1.0)
    abc = np.zeros((NC, 128, 3 * SB), np.float32)
    dv = dinv.reshape(NC, SB, 128)
    abc[:, :, 0:SB] = (0.9 * dv).transpose(0, 2, 1)
    abc[:, :, SB : 2 * SB] = (0.9 * dv * dv).transpose(0, 2, 1)
    abc[:, :, 2 * SB : 3 * SB] = dv.transpose(0, 2, 1)

    # --- weights / x ---
    W1p = np.zeros((FPAD, H), np.float32)
    W1p[:F] = W1
    w1t = (
        W1p.reshape(KT, 128, H).transpose(1, 0, 2).reshape(128, KT * H)
    ).astype(ml_dtypes.bfloat16)
    w2b = W2.astype(ml_dtypes.bfloat16)
    b1c = b1.reshape(H, 1).astype(np.float32)
    b2b = np.tile(b2.reshape(1, C), (128, 1)).astype(np.float32)
    iota = np.tile(
        np.tile(np.arange(W, dtype=np.float32), TT_B).reshape(1, TT_B * W), (128, 1)
    ).astype(ml_dtypes.bfloat16)

    in_maps = []
    for c in range(NC):  # noqa: B007
        sl = slice(c * NLOC, (c + 1) * NLOC)
        xp = np.zeros((NLOC, FPAD), np.float32)
        oc = orig_of[sl]
        m = oc >= 0
        xp[m, :F] = x[oc[m]]
        in_maps.append(
            {
                "xT": np.ascontiguousarray(xp.T).astype(ml_dtypes.bfloat16),
                "w1t": w1t,
                "w2b": w2b,
                "b1c": b1c,
                "b2b": b2b,
                "abc": abc[c],
                "idxs": np.tile(idxs[c], (8, 1)),
                "dstrel": dstrel[c].astype(ml_dtypes.bfloat16),
                "iota": iota,
            }
        )
    return in_maps, orig_of, call_nidx


# ----------------------------------------------------------- device build ----
def build(call_nidx=None):
    nc = bacc.Bacc("TRN2", target_bir_lowering=False, debug=False, num_devices=NC)
    CPC = GPC * NCHUNK
    COLS = QT * 128 // 16
    NIDX = QT * 128
    # Shrunk per-call gathers (num_idxs = max real count over cores) measured
    # ~5% faster but showed rare nondeterministic corruption — suspected DMA
    # completion-semaphore undercount when few descriptors spread over <16
    # SDMA engines. Full-width calls are the validated-stable configuration.
    call_nidx = np.full(CPC, NIDX, np.int64)
    xT = nc.dram_tensor("xT", [FPAD, NLOC], BF16, kind="ExternalInput")
    w1t_d = nc.dram_tensor("w1t", [128, KT * H], BF16, kind="ExternalInput")
    w2b_d = nc.dram_tensor("w2b", [H, C], BF16, kind="ExternalInput")
    b1c_d = nc.dram_tensor("b1c", [H, 1], F32, kind="ExternalInput")
    b2b_d = nc.dram_tensor("b2b", [128, C], F32, kind="ExternalInput")
    abc_d = nc.dram_tensor("abc", [128, 3 * SB], F32, kind="ExternalInput")
    idxs_d = nc.dram_tensor("idxs", [128, CPC * COLS], I16, kind="ExternalInput")
    dstrel_d = nc.dram_tensor("dstrel", [128, GPC * TPG], BF16, kind="ExternalInput")
    iota_d = nc.dram_tensor("iota", [128, TT_B * W], BF16, kind="ExternalInput")
    out_d = nc.dram_tensor("out", [NLOC, C], F32, kind="ExternalOutput")

    AF = mybir.ActivationFunctionType
    OP = mybir.AluOpType

    with tile.TileContext(nc) as tc:
        with (
            tc.tile_pool(name="res", bufs=1) as res,
            tc.tile_pool(name="dram", bufs=1, space="DRAM") as dram,
            tc.tile_pool(name="msgp", bufs=8) as msgp,
            tc.tile_pool(name="stp", bufs=3) as stp,
            tc.tile_pool(name="psA", bufs=2, space="PSUM") as psA_p,
            tc.tile_pool(name="psB", bufs=4, space="PSUM") as psB_p,
            tc.tile_pool(name="xtp", bufs=4) as xtp,
            tc.tile_pool(name="tmp", bufs=8) as tmpp,
            tc.tile_pool(name="zt", bufs=1) as ztp,
        ):
            idxs_t = res.tile([128, CPC * COLS], I16)
            nc.sync.dma_start(out=idxs_t[:], in_=idxs_d[:])
            dstrel_t = res.tile([128, GPC * TPG], BF16)
            nc.sync.dma_start(out=dstrel_t[:], in_=dstrel_d[:])
            iota_t = res.tile([128, TT_B * W], BF16)
            nc.sync.dma_start(out=iota_t[:], in_=iota_d[:])
            w1t_t = res.tile([128, KT * H], BF16)
            nc.sync.dma_start(out=w1t_t[:], in_=w1t_d[:])
            w2b_t = res.tile([H, C], BF16)
            nc.sync.dma_start(out=w2b_t[:], in_=w2b_d[:])
            b1c_t = res.tile([H, 1], F32)
            nc.sync.dma_start(out=b1c_t[:], in_=b1c_d[:])
            b2b_t = res.tile([128, C], F32)
            nc.sync.dma_start(out=b2b_t[:], in_=b2b_d[:])
            abc_t = res.tile([128, 3 * SB], F32)
            nc.sync.dma_start(out=abc_t[:], in_=abc_d[:])

            h_t = res.tile([128, SB * C], F32)
            t0_t = res.tile([128, SB * C], F32)
            gc_t = res.tile([128, SB * C], BF16)

            gin = [dram.tile([NLOC, GROW], BF16, name=f"gin{i}") for i in range(2)]
            gfull = [dram.tile([NG, GROW], BF16, name=f"gfull{i}") for i in range(2)]

            def a_ap(sb):
                return abc_t[:, sb : sb + 1]

            def b_ap(sb):
                return abc_t[:, SB + sb : SB + sb + 1]

            def c_ap(sb):
                return abc_t[:, 2 * SB + sb : 2 * SB + sb + 1]

            def h_sb(sb):
                return h_t[:, sb * C : (sb + 1) * C]

            def t0_sb(sb):
                return t0_t[:, sb * C : (sb + 1) * C]

            def gc_sb(sb):
                return gc_t[:, sb * C : (sb + 1) * C]

            def send_g(it):
                # gc [128, SB*C] -> gin rows (strided 256B rows), then AllGather
                nc.sync.dma_start(
                    out=gin[it % 2][:, :C].rearrange("(s p) f -> p s f", p=128),
                    in_=gc_t[:].rearrange("p (s f) -> p s f", f=C),
                )
                nc.gpsimd.collective_compute(
                    "AllGather",
                    OP.bypass,
                    replica_groups=[list(range(NC))],
                    ins=[gin[it % 2][:].opt()],
                    outs=[gfull[it % 2][:].opt()],
                )

            # ---- MLP stage A ----
            zT = ztp.tile([H, NLOC], BF16)
            for chn in range(NLOC // 512):
                ps = psA_p.tile([H, 512], F32, tag="psA", space="PSUM")
                for k in range(KT):
                    xt = xtp.tile([128, 512], BF16, tag="xt")
                    nc.sync.dma_start(
                        out=xt[:],
                        in_=xT[k * 128 : (k + 1) * 128, chn * 512 : (chn + 1) * 512],
                    )
                    nc.tensor.matmul(
                        out=ps[:],
                        lhsT=w1t_t[:, k * H : (k + 1) * H],
                        rhs=xt[:],
                        start=(k == 0),
                        stop=(k == KT - 1),
                    )
                nc.scalar.activation(
                    out=zT[:, chn * 512 : (chn + 1) * 512],
                    in_=ps[:],
                    func=AF.Relu,
                    bias=b1c_t[:],
                )

            # ---- MLP stage B ----
            for sb in range(SB):
                ps = psB_p.tile([128, C], F32, tag="psB", space="PSUM")
                nc.tensor.matmul(
                    out=ps[:],
                    lhsT=zT[:, sb * 128 : (sb + 1) * 128],
                    rhs=w2b_t[:],
                    start=True,
                    stop=True,
                )
                nc.vector.tensor_add(out=h_sb(sb), in0=ps[:], in1=b2b_t[:])
                nc.vector.tensor_scalar_mul(out=t0_sb(sb), in0=h_sb(sb), scalar1=ALPHA)
                nc.vector.tensor_scalar_mul(
                    out=gc_sb(sb), in0=h_sb(sb), scalar1=c_ap(sb)
                )
            send_g(0)

            # ---- propagation ----
            for it in range(KSTEPS):
                gf = gfull[it % 2]
                for g2 in range(GPC // 2):  # pairs of groups -> one super-block
                    sb = g2  # 2 groups of 64 = 128 dsts = super-block g2
                    ps = psB_p.tile([128, C], F32, tag="psB", space="PSUM")
                    for q2 in range(2):
                        g = g2 * 2 + q2
                        # one-hot build for this group's TPG tiles (20) in
                        # batches of TT_B+4... build 20 tiles in 2 DVE calls
                        st = stp.tile([128, TPG * W], BF16, tag="st")
                        half = TPG // 2  # 10
                        for bi in range(2):
                            t0i = bi * half
                            nc.vector.tensor_tensor(
                                out=st[:, t0i * W : (t0i + half) * W].rearrange(
                                    "p (t w) -> p t w", w=W
                                ),
                                in0=dstrel_t[
                                    :, g * TPG + t0i : g * TPG + t0i + half
                                ].to_broadcast((128, half, W)),
                                in1=iota_t[:, : half * W].rearrange(
                                    "p (t w) -> p t w", w=W
                                ),
                                op=OP.is_equal,
                            )
                        # per-chunk used tile counts; only emitted tiles are
                        # ever gathered or consumed (tiles beyond a call's
                        # width are all-pad on every core by construction)
                        tus = [
                            (int(call_nidx[g * NCHUNK + chk]) + 127) // 128
                            for chk in range(NCHUNK)
                        ]
                        n_mm = sum(tus)
                        mi = 0
                        for chk in range(NCHUNK):
                            call = g * NCHUNK + chk
                            nid = int(call_nidx[call])
                            tu = tus[chk]
                            msg = msgp.tile([128, QT, C], BF16, tag="msg")
                            dma_gather_raw(
                                nc.gpsimd,
                                msg[:, :tu, :],
                                gf[chk * CHUNK : (chk + 1) * CHUNK, :C],
                                idxs_t[
                                    :, call * COLS : call * COLS + nid // 16
                                ],
                                nid,
                                C,
                                GROW,
                            )
                            for t in range(tu):
                                tt = chk * QT + t
                                nc.tensor.matmul(
                                    out=ps[q2 * W : (q2 + 1) * W, :],
                                    lhsT=st[:, tt * W : (tt + 1) * W],
                                    rhs=msg[:, t, :],
                                    start=(mi == 0),
                                    stop=(mi == n_mm - 1),
                                    tile_position=(0, q2 * W),
                                )
                                mi += 1
                    # blend
                    u1 = tmpp.tile([128, C], F32, tag="u1")
                    nc.scalar.activation(
                        out=u1[:], in_=ps[:], func=AF.Copy, scale=a_ap(sb)
                    )
                    u2 = tmpp.tile([128, C], F32, tag="u2")
                    nc.scalar.activation(
                        out=u2[:], in_=h_sb(sb), func=AF.Copy, scale=b_ap(sb)
                    )
                    nc.vector.tensor_add(out=u1[:], in0=u1[:], in1=u2[:])
                    nc.vector.tensor_add(out=h_sb(sb), in0=u1[:], in1=t0_sb(sb))
                    if it < KSTEPS - 1:
                        nc.vector.tensor_scalar_mul(
                            out=gc_sb(sb), in0=h_sb(sb), scalar1=c_ap(sb)
                        )
                if it < KSTEPS - 1:
                    send_g(it + 1)

            # ---- log_softmax + output ----
            for sb in range(SB):
                nm = tmpp.tile([128, 1], F32, tag="nm")
                nc.vector.tensor_reduce(
                    out=nm[:], in_=h_sb(sb), axis=mybir.AxisListType.X,
                    op=OP.max, negate=True,
                )
                ex = tmpp.tile([128, C], F32, tag="ex")
                ssum = tmpp.tile([128, 1], F32, tag="ssum")
                nc.scalar.activation(
                    out=ex[:], in_=h_sb(sb), func=AF.Exp, bias=nm[:],
                    accum_out=ssum[:],
                )
                lse = tmpp.tile([128, 1], F32, tag="lse")
                nc.scalar.activation(out=lse[:], in_=ssum[:], func=AF.Ln)
                off = tmpp.tile([128, 1], F32, tag="off")
                nc.vector.tensor_tensor(
                    out=off[:], in0=nm[:], in1=lse[:], op=OP.subtract
                )
                ostage = tmpp.tile([128, C], F32, tag="ostage")
                nc.vector.tensor_scalar(
                    out=ostage[:], in0=h_sb(sb), scalar1=off[:], scalar2=None,
                    op0=OP.add,
                )
                nc.sync.dma_start(
                    out=out_d[sb * 128 : (sb + 1) * 128, :], in_=ostage[:]
                )
    nc.finalize()
    return nc


_BUILT = None


def _get_built():
    global _BUILT
    if _BUILT is None:
        _BUILT = build()
    return _BUILT


def kernel(**inputs) -> np.ndarray:
    in_maps, orig_of, call_nidx = _preprocess(
        inputs["x"], inputs["W1"], inputs["b1"], inputs["W2"], inputs["b2"],
        inputs["edge_index"],
    )
    global _BUILT
    if _BUILT is None:
        _BUILT = build(call_nidx)
    nc = _BUILT
    res = run_bass_kernel_spmd(nc, in_maps, core_ids=list(range(NC)))
    out = np.empty((N, C), np.float32)
    for c in range(NC):
        block = res.results[c]["out"]
        oc = orig_of[c * NLOC : (c + 1) * NLOC]
        m = oc >= 0
        out[oc[m]] = block[m]
    return out



# revision 8
# speedup vs baseline: 1.1095x; 1.1095x over previous
"""APPNP GNN kernel for 8 Trainium2 NeuronCores.

h0 = MLP(x); h <- 0.9 * D^-1/2 (A+I) D^-1/2 h + 0.1*h0 (10 steps); log_softmax.

Distribution: nodes permuted + bin-packed into degree-balanced groups of <=64
destinations (<=2560 in-edges), 200 groups/core (12800 slots/core). All cores
run one SPMD program; per-core structure lives in data tables.

The gather table is CHUNK-MAJOR: chunk c holds local rows [3200c,3200(c+1))
of every core, AllGathered separately per chunk so chunk-c gathers overlap
the other chunks' collectives. Per iteration each core:
  - per (group, chunk): dma_gather of bf16 source rows (int16 chunk-relative
    indices, static per-call num_idxs = max real count over cores, >=256),
  - segment-sums messages into a RESIDENT PSUM accumulator (one [128x40]
    region per superblock, accumulated across all 4 chunk passes) with one
    [128x64] one-hot matmul per 128-edge tile,
  - after the last chunk pass, blends with the self-loop term and alpha*h0,
  - per-chunk DMA+AllGather of the rescaled g = dinv*h for the next round,
    emitted as soon as the superblocks feeding that chunk are blended.
Superblocks are processed in 2 halves (PSUM capacity: 5 bank tiles of 10
superblock regions per half). MLP on pre-transposed bf16 x^T; log_softmax on
device. Host preprocessing is numpy-only.
"""

import numpy as np
import ml_dtypes

import concourse.bass as bass
import concourse.bacc as bacc
import concourse.tile as tile
import concourse.mybir as mybir
from concourse import ap_utils
from concourse.bass import MemorySpace
from concourse.bass_utils import run_bass_kernel_spmd

# ---------------------------------------------------------------- config ----
NC = 8          # cores
N = 100000      # nodes
F = 500         # input features
H = 64          # hidden
C = 40          # classes
E = 3200000     # edges
KSTEPS = 10
ALPHA = 0.1

W = 64          # dst nodes per group
GPC = 200       # groups per core
QT = 5          # 128-slot tiles per (group, chunk) block
NCHUNK = 4      # int16 src chunks
NLOC = GPC * W               # node slots per core (12800)
SB = NLOC // 128             # super-blocks per core (100)
NG = NC * NLOC               # gather-table rows (102400)
CHUNK = NG // NCHUNK         # rows per chunk (25600)
FPAD = 512
KT = FPAD // 128
GROW = 128                   # padded g row (elements, bf16) => 256B stride
HALF = 2                     # superblock halves (PSUM capacity)
MIN_NIDX = 256               # static num_idxs floor (desc-spread safety)

BF16 = mybir.dt.bfloat16
F32 = mybir.dt.float32
I16 = mybir.dt.int16


def exact_div(a, b):
    assert a % b == 0
    return a // b


def dma_gather_raw(gp, out_ap, in_ap, idxs_ap, num_idxs, elem_size, elem_step,
                   num_idxs_reg=None):
    """dma_gather without the elem_size%256 restriction (stride still 256B).

    num_idxs_reg is the live descriptor count on the Q7 (min-capped by the
    static num_idxs); it must equal the number of valid leading indices."""
    if num_idxs_reg is None:
        num_idxs_reg = num_idxs
    assert idxs_ap.dtype == mybir.dt.int16
    assert in_ap.dtype == out_ap.dtype
    assert in_ap.space == MemorySpace.DRAM
    assert idxs_ap.space == MemorySpace.SBUF
    assert out_ap.space == MemorySpace.SBUF
    assert ap_utils.ap_is_contiguous(out_ap.ap[1:])
    assert ap_utils.ap_is_contiguous(idxs_ap.ap[1:])
    assert in_ap.ap[-1][1] == out_ap.ap[-1][1] == elem_size
    assert out_ap.ap[0][1] * out_ap.ap[1][1] == ((num_idxs + 127) // 128) * 128
    assert in_ap.ap[0][0] == elem_step
    stride_bytes = elem_step * mybir.dt.size(in_ap.dtype)
    stride_bytes_256 = exact_div(stride_bytes, 256)
    _in_ap = gp.lower_ap_dma(in_ap, for_custom_bir_dma=True)
    _idxs_ap = gp.lower_ap(idxs_ap)
    _out_ap = gp.lower_ap(out_ap)
    return gp.add_instruction(
        mybir.InstDMAGatherAnt(
            name=gp.bass.get_next_instruction_name(),
            ins=[*_in_ap, _idxs_ap, gp.lower_val_access(gp.to_reg(num_idxs_reg))],
            outs=[_out_ap],
            transpose=False,
            num_idxs=num_idxs,
            elem_size=elem_size,
            stride_bytes_256=stride_bytes_256,
            gen_mode=0,
            single_packet=True,
            queue_num=0,
            sbuf_tokens_per_rank=0,
            sbuf_free_dim_per_rank=0,
            sbuf_free_dim_pad_per_rank=0,
            sbuf_byte_offset=0,
        )
    )


# ---------------------------------------------------------- preprocessing ----
def _pack_groups(deg, n_groups, cap_nodes, cap_edges):
    import heapq

    n = deg.shape[0]
    order = np.argsort(-deg, kind="stable")
    heap = [(0, g) for g in range(n_groups)]
    heapq.heapify(heap)
    nodes_in = np.zeros(n_groups, np.int64)
    group_of = np.empty(n, np.int64)
    pos_of = np.empty(n, np.int64)
    for node in order:
        d = int(deg[node])
        while True:
            if not heap:
                raise RuntimeError("group packing failed")
            esum, g = heapq.heappop(heap)
            if nodes_in[g] >= cap_nodes:
                continue
            if esum + d > cap_edges:
                raise RuntimeError(f"packing: min sum {esum} + {d} > {cap_edges}")
            group_of[node] = g
            pos_of[node] = nodes_in[g]
            nodes_in[g] += 1
            heapq.heappush(heap, (esum + d, g))
            break
    return group_of, pos_of


def _preprocess(x, W1, b1, W2, b2, edge_index):
    CPC = GPC * NCHUNK
    COLS = QT * 128 // 16
    LPC = NLOC // NCHUNK
    x = np.asarray(x, np.float32)
    W1 = np.asarray(W1, np.float32)
    b1 = np.asarray(b1, np.float32)
    W2 = np.asarray(W2, np.float32)
    b2 = np.asarray(b2, np.float32)
    ei = np.asarray(edge_index)
    src, dst = ei[0].astype(np.int64), ei[1].astype(np.int64)

    deg = np.bincount(dst, minlength=N).astype(np.int64)
    group_of, pos_of = _pack_groups(deg, NC * GPC, W, QT * 128 * NCHUNK)
    new_id = group_of * W + pos_of
    orig_of = np.full(NG, -1, np.int64)
    orig_of[new_id] = np.arange(N)

    # --- edge -> slot tables ---
    src_n = new_id[src]
    dst_n = new_id[dst]
    g_e = dst_n // W                       # global group
    w_e = (dst_n % W).astype(np.float32)   # within-group dst index
    # chunk-major gather table: chunk of a source = its local-row quarter;
    # within chunk c rows are [core0 rows, core1 rows, ...] (LPC each)
    core_s = src_n // NLOC
    loc_s = src_n % NLOC
    c_e = loc_s // LPC                     # src chunk
    rel_e = core_s * LPC + (loc_s % LPC)   # row within chunk [0, CHUNK)
    key = g_e * NCHUNK + c_e               # global (group, chunk) id
    order = np.argsort(key, kind="stable")
    key_s = key[order]
    rel_s = rel_e[order]
    w_s = w_e[order]
    starts = np.searchsorted(key_s, np.arange(NC * GPC * NCHUNK))
    pos = np.arange(rel_s.shape[0]) - starts[key_s]
    assert pos.max() < QT * 128, f"chunk-block overflow: {pos.max()}"
    gl = (key_s // NCHUNK) % GPC
    core_e = key_s // (GPC * NCHUNK)
    ch = key_s % NCHUNK
    call = gl * NCHUNK + ch                # per-core call id [0, CPC)
    lane = pos % 128

    # dstrel table [core][128][CPC*QT]  (indexed (call, tile))
    dstrel = np.full((NC, 128, CPC * QT), -1.0, np.float32)
    dstrel[core_e, lane, call * QT + pos // 128] = w_s

    # idx table: per call (group, chunk) wrapped int16 [16, CPC*COLS] -> x8
    idxs = np.zeros((NC, 16, CPC * COLS), np.int16)
    idxs[core_e, pos % 16, call * COLS + pos // 16] = rel_s.astype(np.int16)
    # static per-call index count: max real count over cores (SPMD => one
    # program), rounded to 16, floored at MIN_NIDX for SDMA desc spread
    gcnt = np.zeros((NC, CPC), np.int64)
    np.add.at(gcnt, (core_e, call), 1)
    call_nidx = np.clip(
        (gcnt.max(axis=0) + 15) // 16 * 16, min(MIN_NIDX, QT * 128), QT * 128
    ).astype(np.int64)

    # --- per-node scalars packed [128, 3*SB] ---
    dinv = np.zeros(NG, np.float32)
    real = orig_of >= 0
    dinv[real] = 1.0 / np.sqrt(deg[orig_of[real]] + 1.0)
    abc = np.zeros((NC, 128, 3 * SB), np.float32)
    dv = dinv.reshape(NC, SB, 128)
    abc[:, :, 0:SB] = (0.9 * dv).transpose(0, 2, 1)
    abc[:, :, SB : 2 * SB] = (0.9 * dv * dv).transpose(0, 2, 1)
    abc[:, :, 2 * SB : 3 * SB] = dv.transpose(0, 2, 1)

    # --- weights / x ---
    W1p = np.zeros((FPAD, H), np.float32)
    W1p[:F] = W1
    w1t = (
        W1p.reshape(KT, 128, H).transpose(1, 0, 2).reshape(128, KT * H)
    ).astype(ml_dtypes.bfloat16)
    w2b = W2.astype(ml_dtypes.bfloat16)
    b1c = b1.reshape(H, 1).astype(np.float32)
    b2b = np.tile(b2.reshape(1, C), (128, 1)).astype(np.float32)
    iota = np.tile(
        np.tile(np.arange(W, dtype=np.float32), QT).reshape(1, QT * W), (128, 1)
    ).astype(ml_dtypes.bfloat16)

    in_maps = []
    for c in range(NC):  # noqa: B007
        sl = slice(c * NLOC, (c + 1) * NLOC)
        xp = np.zeros((NLOC, FPAD), np.float32)
        oc = orig_of[sl]
        m = oc >= 0
        xp[m, :F] = x[oc[m]]
        in_maps.append(
            {
                "xT": np.ascontiguousarray(xp.T).astype(ml_dtypes.bfloat16),
                "w1t": w1t,
                "w2b": w2b,
                "b1c": b1c,
                "b2b": b2b,
                "abc": abc[c],
                "idxs": np.tile(idxs[c], (8, 1)),
                "dstrel": dstrel[c].astype(ml_dtypes.bfloat16),
                "iota": iota,
            }
        )
    return in_maps, orig_of, call_nidx


# ----------------------------------------------------------- device build ----
def build(call_nidx=None):
    CPC = GPC * NCHUNK
    COLS = QT * 128 // 16
    SBC = SB // NCHUNK           # superblocks feeding one chunk
    SBH = SB // HALF             # superblocks per half
    PSB = min(10, SBH)           # superblock regions per PSUM bank tile
    assert SBH % PSB == 0
    nc = bacc.Bacc("TRN2", target_bir_lowering=False, debug=False, num_devices=NC)
    if call_nidx is None:
        call_nidx = np.full(CPC, QT * 128, np.int64)
    tu_of = [(int(call_nidx[k]) + 127) // 128 for k in range(CPC)]

    xT = nc.dram_tensor("xT", [FPAD, NLOC], BF16, kind="ExternalInput")
    w1t_d = nc.dram_tensor("w1t", [128, KT * H], BF16, kind="ExternalInput")
    w2b_d = nc.dram_tensor("w2b", [H, C], BF16, kind="ExternalInput")
    b1c_d = nc.dram_tensor("b1c", [H, 1], F32, kind="ExternalInput")
    b2b_d = nc.dram_tensor("b2b", [128, C], F32, kind="ExternalInput")
    abc_d = nc.dram_tensor("abc", [128, 3 * SB], F32, kind="ExternalInput")
    idxs_d = nc.dram_tensor("idxs", [128, CPC * COLS], I16, kind="ExternalInput")
    dstrel_d = nc.dram_tensor("dstrel", [128, CPC * QT], BF16, kind="ExternalInput")
    iota_d = nc.dram_tensor("iota", [128, QT * W], BF16, kind="ExternalInput")
    out_d = nc.dram_tensor("out", [NLOC, C], F32, kind="ExternalOutput")

    AF = mybir.ActivationFunctionType
    OP = mybir.AluOpType

    with tile.TileContext(nc) as tc:
        with (
            tc.tile_pool(name="res", bufs=1) as res,
            tc.tile_pool(name="dram", bufs=1, space="DRAM") as dram,
            tc.tile_pool(name="msgp", bufs=8) as msgp,
            tc.tile_pool(name="stp", bufs=4) as stp,
            tc.tile_pool(name="xtp", bufs=4) as xtp,
            tc.tile_pool(name="tmp", bufs=8) as tmpp,
            tc.tile_pool(name="zt", bufs=1) as ztp,
        ):
            idxs_t = res.tile([128, CPC * COLS], I16)
            nc.sync.dma_start(out=idxs_t[:], in_=idxs_d[:])
            dstrel_t = res.tile([128, CPC * QT], BF16)
            nc.sync.dma_start(out=dstrel_t[:], in_=dstrel_d[:])
            iota_t = res.tile([128, QT * W], BF16)
            nc.sync.dma_start(out=iota_t[:], in_=iota_d[:])
            w1t_t = res.tile([128, KT * H], BF16)
            nc.sync.dma_start(out=w1t_t[:], in_=w1t_d[:])
            w2b_t = res.tile([H, C], BF16)
            nc.sync.dma_start(out=w2b_t[:], in_=w2b_d[:])
            b1c_t = res.tile([H, 1], F32)
            nc.sync.dma_start(out=b1c_t[:], in_=b1c_d[:])
            b2b_t = res.tile([128, C], F32)
            nc.sync.dma_start(out=b2b_t[:], in_=b2b_d[:])
            abc_t = res.tile([128, 3 * SB], F32)
            nc.sync.dma_start(out=abc_t[:], in_=abc_d[:])

            h_t = res.tile([128, SB * C], F32)
            t0_t = res.tile([128, SB * C], F32)
            gc_t = res.tile([128, SB * C], BF16)
            agg_t = res.tile([128, SB * C], F32)

            gin = [
                dram.tile([NLOC // NCHUNK, GROW], BF16, name=f"gin{c}")
                for c in range(NCHUNK)
            ]
            gch = [
                [dram.tile([CHUNK, GROW], BF16, name=f"gch{c}_{p}")
                 for p in range(2)]
                for c in range(NCHUNK)
            ]

            def a_ap(sb):
                return abc_t[:, sb : sb + 1]

            def b_ap(sb):
                return abc_t[:, SB + sb : SB + sb + 1]

            def c_ap(sb):
                return abc_t[:, 2 * SB + sb : 2 * SB + sb + 1]

            def h_sb(sb):
                return h_t[:, sb * C : (sb + 1) * C]

            def t0_sb(sb):
                return t0_t[:, sb * C : (sb + 1) * C]

            def gc_sb(sb):
                return gc_t[:, sb * C : (sb + 1) * C]

            def send_chunk(cidx, parity):
                # gc superblocks [SBC*cidx, SBC*(cidx+1)) -> gin rows -> AllGather
                s0 = SBC * cidx
                nc.sync.dma_start(
                    out=gin[cidx][:, :C].rearrange("(s p) f -> p s f", p=128),
                    in_=gc_t[:, s0 * C : (s0 + SBC) * C].rearrange(
                        "p (s f) -> p s f", f=C
                    ),
                )
                nc.gpsimd.collective_compute(
                    "AllGather",
                    OP.bypass,
                    replica_groups=[list(range(NC))],
                    ins=[gin[cidx][:].opt()],
                    outs=[gch[cidx][parity][:].opt()],
                )

            # ---- MLP (scoped PSUM pools; freed before propagation) ----
            with (
                tc.tile_pool(name="psA", bufs=2, space="PSUM") as psA_p,
                tc.tile_pool(name="psB", bufs=4, space="PSUM") as psB_p,
            ):
                zT = ztp.tile([H, NLOC], BF16)
                for chn in range(NLOC // 512):
                    ps = psA_p.tile([H, 512], F32, tag="psA", space="PSUM")
                    for k in range(KT):
                        xt = xtp.tile([128, 512], BF16, tag="xt")
                        nc.sync.dma_start(
                            out=xt[:],
                            in_=xT[k * 128 : (k + 1) * 128,
                                   chn * 512 : (chn + 1) * 512],
                        )
                        nc.tensor.matmul(
                            out=ps[:],
                            lhsT=w1t_t[:, k * H : (k + 1) * H],
                            rhs=xt[:],
                            start=(k == 0),
                            stop=(k == KT - 1),
                        )
                    nc.scalar.activation(
                        out=zT[:, chn * 512 : (chn + 1) * 512],
                        in_=ps[:],
                        func=AF.Relu,
                        bias=b1c_t[:],
                    )

                for sb in range(SB):
                    ps = psB_p.tile([128, C], F32, tag="psB", space="PSUM")
                    nc.tensor.matmul(
                        out=ps[:],
                        lhsT=zT[:, sb * 128 : (sb + 1) * 128],
                        rhs=w2b_t[:],
                        start=True,
                        stop=True,
                    )
                    nc.vector.tensor_add(out=h_sb(sb), in0=ps[:], in1=b2b_t[:])
                    nc.vector.tensor_scalar_mul(
                        out=t0_sb(sb), in0=h_sb(sb), scalar1=ALPHA
                    )
                    nc.scalar.activation(
                        out=gc_sb(sb), in_=h_sb(sb), func=AF.Copy, scale=c_ap(sb)
                    )
                    if (sb + 1) % SBC == 0:
                        send_chunk(sb // SBC, 0)

            # ---- propagation ----
            with tc.tile_pool(name="psP", bufs=4, space="PSUM") as psP_p:
                for it in range(KSTEPS):
                    parity = it % 2
                    for chk in range(NCHUNK):
                        gf = gch[chk][parity]
                        for sb in range(SB):
                            ps = psP_p.tile([128, C], F32, tag="psP",
                                            space="PSUM", name="ps")
                            for q2 in range(2):
                                g = sb * 2 + q2
                                call = g * NCHUNK + chk
                                nid = int(call_nidx[call])
                                tu = tu_of[call]
                                st = stp.tile([128, QT * W], BF16, tag="st")
                                nc.vector.tensor_tensor(
                                    out=st[:, : tu * W].rearrange(
                                        "p (t w) -> p t w", w=W
                                    ),
                                    in0=dstrel_t[
                                        :, call * QT : call * QT + tu
                                    ].to_broadcast((128, tu, W)),
                                    in1=iota_t[:, : tu * W].rearrange(
                                        "p (t w) -> p t w", w=W
                                    ),
                                    op=OP.is_equal,
                                )
                                msg = msgp.tile([128, QT, C], BF16, tag="msg")
                                dma_gather_raw(
                                    nc.gpsimd,
                                    msg[:, :tu, :],
                                    gf[:, :C],
                                    idxs_t[:, call * COLS
                                           : call * COLS + nid // 16],
                                    nid,
                                    C,
                                    GROW,
                                )
                                for t in range(tu):
                                    nc.tensor.matmul(
                                        out=ps[q2 * W : (q2 + 1) * W, :],
                                        lhsT=st[:, t * W : (t + 1) * W],
                                        rhs=msg[:, t, :],
                                        start=(t == 0),
                                        stop=(t == tu - 1),
                                        tile_position=(0, q2 * W),
                                    )
                            ag = agg_t[:, sb * C : (sb + 1) * C]
                            if chk == 0:
                                nc.vector.tensor_copy(out=ag, in_=ps[:])
                            else:
                                nc.vector.tensor_add(out=ag, in0=ag, in1=ps[:])
                            if chk == NCHUNK - 1:
                                u1 = tmpp.tile([128, C], F32, tag="u1")
                                nc.scalar.activation(
                                    out=u1[:], in_=ag, func=AF.Copy,
                                    scale=a_ap(sb),
                                )
                                u2 = tmpp.tile([128, C], F32, tag="u2")
                                nc.scalar.activation(
                                    out=u2[:], in_=h_sb(sb), func=AF.Copy,
                                    scale=b_ap(sb),
                                )
                                nc.vector.tensor_add(
                                    out=u1[:], in0=u1[:], in1=u2[:]
                                )
                                nc.vector.tensor_add(
                                    out=h_sb(sb), in0=u1[:], in1=t0_sb(sb)
                                )
                                if it < KSTEPS - 1:
                                    nc.scalar.activation(
                                        out=gc_sb(sb), in_=h_sb(sb),
                                        func=AF.Copy, scale=c_ap(sb),
                                    )
                                    if (sb + 1) % SBC == 0:
                                        send_chunk(sb // SBC, (it + 1) % 2)

            # ---- log_softmax + output ----
            for sb in range(SB):
                nm = tmpp.tile([128, 1], F32, tag="nm")
                nc.vector.tensor_reduce(
                    out=nm[:], in_=h_sb(sb), axis=mybir.AxisListType.X,
                    op=OP.max, negate=True,
                )
                ex = tmpp.tile([128, C], F32, tag="ex")
                ssum = tmpp.tile([128, 1], F32, tag="ssum")
                nc.scalar.activation(
                    out=ex[:], in_=h_sb(sb), func=AF.Exp, bias=nm[:],
                    accum_out=ssum[:],
                )
                lse = tmpp.tile([128, 1], F32, tag="lse")
                nc.scalar.activation(out=lse[:], in_=ssum[:], func=AF.Ln)
                off = tmpp.tile([128, 1], F32, tag="off")
                nc.vector.tensor_tensor(
                    out=off[:], in0=nm[:], in1=lse[:], op=OP.subtract
                )
                ostage = tmpp.tile([128, C], F32, tag="ostage")
                nc.vector.tensor_scalar(
                    out=ostage[:], in0=h_sb(sb), scalar1=off[:], scalar2=None,
                    op0=OP.add,
                )
                nc.sync.dma_start(
                    out=out_d[sb * 128 : (sb + 1) * 128, :], in_=ostage[:]
                )
    nc.finalize()
    return nc


_BUILT = None


def kernel(**inputs) -> np.ndarray:
    in_maps, orig_of, call_nidx = _preprocess(
        inputs["x"], inputs["W1"], inputs["b1"], inputs["W2"], inputs["b2"],
        inputs["edge_index"],
    )
    global _BUILT
    if _BUILT is None:
        _BUILT = build(call_nidx)
    nc = _BUILT
    res = run_bass_kernel_spmd(nc, in_maps, core_ids=list(range(NC)))
    out = np.empty((N, C), np.float32)
    for c in range(NC):
        block = res.results[c]["out"]
        oc = orig_of[c * NLOC : (c + 1) * NLOC]
        m = oc >= 0
        out[oc[m]] = block[m]
    return out
